# revision 9
# baseline (speedup 1.0000x reference)
"""ComplexLayerNorm Trainium2 kernel (8 NeuronCores, SPMD, C-sharded).

Math (see reference): per-feature 2x2 covariance whitening of (re, im) over
all B*C samples (centered with the batch-only mean mu_b), after subtracting
the complex mean over F, plus complex affine.

Sharding: C (=128) split 16-per-core, so the batch-sums T[c,f] = sum_b x and
mu_b are core-local and only 3 partial second-moment F-vectors (24 KB) need an
AllReduce.

Per-core dataflow:
  1. Load x natural ((bc=1024) x (f=2048)); PE identity-matmuls transpose it
     to f-on-partitions (xT), and selector-matmuls accumulate T per (f, c).
  2. Second moments per f: ACT Square w/ accum_out + DVE tensor_tensor_reduce.
     T-quadratic correction likewise. Partial cov -> AllReduce (24 KB).
  3. Complex mean over F per bc: PE ones-matmul over xT.
  4. Closed-form 2x2 inverse sqrt: s=sqrt(det), t=sqrt(tr+2s),
     M^-1/2 = [[c+s, -b], [-b, a+s]]/(s*t); fold complex gamma -> A (2x2 per f).
  5. Apply on PE: out[bc, 2f+c] = sum_f' xT[f',bc]*W[f',2f+c] with W sparse
     diagonals of A, plus one K=3 matmul adding (-A@mean + beta).
     Output lands interleaved (f, 2) -- exactly the (B,C,F,2) layout.
Host only slices inputs per core and concatenates the 8 C-shards.
"""

import numpy as np

import bass_rust
import concourse.bass as bass
import concourse.mybir as mybir
from concourse import tile
from concourse.bass_utils import run_bass_kernel_spmd


def split_multi_waits(nc):
    """The walrus build in this container allows only ONE sync-wait command
    per instruction; Tile emits several.  Split extras into preceding
    single-wait NoOps on the same engine (sequential waits == AND)."""
    cnt = 0
    for bb in nc.main_func.blocks:
        il = bb.instructions
        newlist = []
        changed = False
        for inst in list(il):
            si = inst.sync_info
            waits = list(si.on_wait) if si else []
            if len(waits) > 1:
                changed = True
                for w in waits[:-1]:
                    cnt += 1
                    nop = bass_rust.InstNoOp(name=f"I-wsplit-{cnt}")
                    nop.engine = inst.engine
                    nop.sync_info = mybir.SyncInfo(on_wait=[w], on_update=[])
                    newlist.append(nop)
                inst.sync_info = mybir.SyncInfo(
                    on_wait=[waits[-1]], on_update=list(si.on_update))
            newlist.append(inst)
        if changed:
            il[:] = newlist
    return cnt

FP = mybir.dt.float32
FPR = mybir.dt.float32r
AF = mybir.ActivationFunctionType
OP = mybir.AluOpType


def mmr(nc, out, lhsT, rhs, **kw):
    """Matmul with both operands bitcast to float32r (1 cyc/row when the
    output free dim is >= 256, vs 4 for plain fp32; sim-exact numerics)."""
    return nc.tensor.matmul(out, lhsT.bitcast(FPR), rhs.bitcast(FPR), **kw)

B, C, F = 64, 128, 2048
NCORES = 8
CSH = C // NCORES           # 16 channels per core
BC = B * CSH                # 1024 sample rows per core
NFT = F // 128              # 16 f-chunks
NBC = BC // 128             # 8 bc-chunks
EPS = 1e-4
NM1 = float(B * C - 1)      # 8191


def build_bass():
    nc = bass.Bass()

    x_r = nc.dram_tensor("x_r", [BC, F], FP, kind="ExternalInput")
    x_i = nc.dram_tensor("x_i", [BC, F], FP, kind="ExternalInput")
    # gamma pre-tiled on host to (128, NFT): tile[p, t] = gamma[128*t + p]
    g_r = nc.dram_tensor("g_r", [128, NFT], FP, kind="ExternalInput")
    g_i = nc.dram_tensor("g_i", [128, NFT], FP, kind="ExternalInput")
    # beta interleaved on host: (1, 4096) = [b_r[0], b_i[0], b_r[1], ...]
    beta_ilv = nc.dram_tensor("beta_ilv", [1, 2 * F], FPR, kind="ExternalInput")
    ident = nc.dram_tensor("ident", [128, 128], FP, kind="ExternalInput")
    identsel = nc.dram_tensor("identsel", [128, 128 + CSH], FP,
                              kind="ExternalInput")
    onesF = nc.dram_tensor("onesF", [128, 1], FPR, kind="ExternalInput")
    ones_bc = nc.dram_tensor("ones_bc", [1, BC], FPR, kind="ExternalInput")

    out = nc.dram_tensor("out", [BC, 2 * F], FP, kind="ExternalOutput")

    with tile.TileContext(nc) as tc:
        with (
            tc.tile_pool(name="big", bufs=1) as big,
            tc.tile_pool(name="small", bufs=1) as small,
            tc.tile_pool(name="wpool", bufs=2) as wpool,
            tc.tile_pool(name="stage", bufs=2) as stage,
            tc.tile_pool(name="dram", bufs=1, space="DRAM") as dram,
        ):
            # ---- constants to SBUF
            ident_t = small.tile([128, 128], FP, tag="ident")
            nc.sync.dma_start(ident_t[:], ident[:])
            identsel_t = small.tile([128, 128 + CSH], FP, tag="identsel")
            nc.sync.dma_start(identsel_t[:], identsel[:])
            onesF_t = small.tile([128, 1], FPR, tag="onesF")
            nc.sync.dma_start(onesF_t[:], onesF[:])
            g_r_t = small.tile([128, NFT], FP, tag="g_r")
            nc.sync.dma_start(g_r_t[:], g_r[:])
            g_i_t = small.tile([128, NFT], FP, tag="g_i")
            nc.sync.dma_start(g_i_t[:], g_i[:])

            # ---- persistent: x transposed, xT[p, 1024*t + j] = x[j, 128*t+p]
            xT_r = big.tile([128, NFT * BC], FPR, tag="xT_r")
            xT_i = big.tile([128, NFT * BC], FPR, tag="xT_i")

            from contextlib import ExitStack
            _stk = ExitStack()
            xin = _stk.enter_context(tc.tile_pool(name="xin", bufs=2))
            scratch = _stk.enter_context(tc.tile_pool(name="scratch", bufs=2))
            ps_xt = _stk.enter_context(
                tc.tile_pool(name="ps_xt", bufs=2, space="PSUM"))

            # T[c,f] accumulators in SBUF: [p (=f in chunk), 16*t + c].
            # (Interleaved long-lived PSUM accumulation groups in one bank
            # miscompile, so accumulate per-b PSUM results on DVE instead.)
            T_r_sb = small.tile([128, NFT * CSH], FP, tag="T_r_sb")
            T_i_sb = small.tile([128, NFT * CSH], FP, tag="T_i_sb")

            # ---- Phase A: load natural; PE transpose + T accumulate
            copy_flip = 0
            for b in range(NBC):
                # split the two loads across the two HWDGE rings (SP / ACT)
                xn_r = xin.tile([128, F], FP, tag="xn")
                nc.sync.dma_start(xn_r[:], x_r[128 * b:128 * (b + 1), :])
                xn_i = xin.tile([128, F], FP, tag="xn")
                nc.scalar.dma_start(xn_i[:], x_i[128 * b:128 * (b + 1), :])
                for xn, xT, T_sb in (
                    (xn_r, xT_r, T_r_sb), (xn_i, xT_i, T_i_sb)
                ):
                    for tg in range(NFT // 4):
                        # one matmul per f-chunk: rhs [ident | sel] gives the
                        # 128-col transpose AND the 16-col T partial. regions
                        # at 256-col spacing so no mm output crosses a bank.
                        pxt = ps_xt.tile([128, 1024], FP, tag="pxt")
                        for tt in range(4):
                            t = 4 * tg + tt
                            nc.tensor.matmul(
                                pxt[:, 256 * tt:256 * tt + 128 + CSH],
                                xn[:, 128 * t:128 * (t + 1)],
                                identsel_t[:],
                                start=True, stop=True,
                            )
                        pv = pxt[:].rearrange("p (a q) -> p a q", q=256)
                        dst = xT[:].rearrange("p (a q) -> p a q", q=1024)[
                            :, 4 * tg:4 * (tg + 1), 128 * b:128 * (b + 1)
                        ]
                        if copy_flip % 2 == 0:
                            nc.vector.tensor_copy(dst, pv[:, :, 0:128])
                        else:
                            nc.scalar.copy(dst, pv[:, :, 0:128])
                        copy_flip += 1
                        tdst = T_sb[:, 64 * tg:64 * (tg + 1)].rearrange(
                            "p (a q) -> p a q", q=CSH)
                        tsrc = pv[:, :, 128:128 + CSH]
                        if b == 0:
                            nc.vector.tensor_copy(tdst, tsrc)
                        else:
                            nc.vector.scalar_tensor_tensor(
                                out=tdst, in0=tsrc, scalar=1.0, in1=tdst,
                                op0=OP.mult, op1=OP.add,
                            )

            # ---- Phase A2: second moments per f-chunk
            S_rr = small.tile([128, NFT], FP, tag="S_rr")
            S_ri = small.tile([128, NFT], FP, tag="S_ri")
            S_ii = small.tile([128, NFT], FP, tag="S_ii")
            for t in range(NFT):
                sl = slice(BC * t, BC * (t + 1))
                sc1 = scratch.tile([128, BC], FP, tag="sq")
                nc.scalar.activation(sc1[:], xT_r[:, sl], AF.Square,
                                     accum_out=S_rr[:, t:t + 1])
                sc2 = scratch.tile([128, BC], FP, tag="sq")
                nc.scalar.activation(sc2[:], xT_i[:, sl], AF.Square,
                                     accum_out=S_ii[:, t:t + 1])
                sc3 = scratch.tile([128, BC], FP, tag="sq")
                nc.vector.scalar_tensor_tensor(
                    out=sc3[:], in0=xT_r[:, sl], scalar=1.0,
                    in1=xT_i[:, sl], op0=OP.mult, op1=OP.mult,
                    accum_out=S_ri[:, t:t + 1],
                )

            # ---- T quadratic correction: corr_xy[:, t] = sum_c T_x*T_y
            corr_rr = small.tile([128, NFT], FP, tag="corr_rr")
            corr_ri = small.tile([128, NFT], FP, tag="corr_ri")
            corr_ii = small.tile([128, NFT], FP, tag="corr_ii")
            for t in range(NFT):
                sl = slice(CSH * t, CSH * (t + 1))
                ts1 = scratch.tile([128, CSH], FP, tag="tsq")
                nc.scalar.activation(ts1[:], T_r_sb[:, sl], AF.Square,
                                     accum_out=corr_rr[:, t:t + 1])
                ts2 = scratch.tile([128, CSH], FP, tag="tsq")
                nc.scalar.activation(ts2[:], T_i_sb[:, sl], AF.Square,
                                     accum_out=corr_ii[:, t:t + 1])
                ts3 = scratch.tile([128, CSH], FP, tag="tsq")
                nc.vector.scalar_tensor_tensor(
                    out=ts3[:], in0=T_r_sb[:, sl], scalar=1.0,
                    in1=T_i_sb[:, sl], op0=OP.mult, op1=OP.mult,
                    accum_out=corr_ri[:, t:t + 1],
                )

            # ---- local partial covariance: (S - corr/B) / (n-1), packed
            partial = small.tile([128, 3 * NFT], FP, tag="partial")
            for j, (S, corr) in enumerate(
                ((S_rr, corr_rr), (S_ri, corr_ri), (S_ii, corr_ii))
            ):
                dst = partial[:, NFT * j:NFT * (j + 1)]
                nc.vector.scalar_tensor_tensor(
                    out=dst, in0=corr[:], scalar=-1.0 / B, in1=S[:],
                    op0=OP.mult, op1=OP.add,
                )
                nc.vector.tensor_scalar(
                    out=dst, in0=dst, scalar1=1.0 / NM1, scalar2=None,
                    op0=OP.mult,
                )

            # ---- AllReduce partial covariance (24 KB)
            ar_in = dram.tile([128, 3 * NFT], FP, tag="ar_in")
            ar_out = dram.tile([128, 3 * NFT], FP, tag="ar_out")
            nc.sync.dma_start(ar_in[:], partial[:])
            nc.gpsimd.collective_compute(
                "AllReduce", OP.add,
                replica_groups=[list(range(NCORES))],
                ins=[ar_in.opt()],
                outs=[ar_out.opt()],
            )
            cov = small.tile([128, 3 * NFT], FP, tag="cov")
            nc.sync.dma_start(cov[:], ar_out[:])

            # release phase-A pools (xin/scratch SBUF, transpose/T PSUM)
            _stk.close()

            # ---- Phase B: complex mean over F via PE ones-matmul on xT
            _stk2 = ExitStack()
            ps_mean = _stk2.enter_context(
                tc.tile_pool(name="ps_mean", bufs=1, space="PSUM"))
            psm_r = ps_mean.tile([1, BC], FP, tag="psm_r")
            psm_i = ps_mean.tile([1, BC], FP, tag="psm_i")
            for xT, psm in ((xT_r, psm_r), (xT_i, psm_i)):
                for t in range(NFT):
                    for h in range(2):
                        mmr(
                            nc,
                            psm[:, 512 * h:512 * (h + 1)],
                            onesF_t[:],
                            xT[:, BC * t + 512 * h:BC * t + 512 * (h + 1)],
                            start=(t == 0), stop=(t == NFT - 1),
                        )
            # M3 = [-mean_r; -mean_i; ones]  (3, 1024).  Engine ops cannot
            # write at partition offsets 1/2, so build rows at partition 0
            # and DMA them into place.
            M3 = small.tile([3, BC], FPR, tag="M3")
            row0 = small.tile([1, BC], FPR, tag="rowtmp", name="row0")
            nc.vector.tensor_scalar(out=row0[:], in0=psm_r[:],
                                    scalar1=-1.0, scalar2=None, op0=OP.mult)
            nc.sync.dma_start(M3[0:1, :], row0[:])
            row1 = small.tile([1, BC], FPR, tag="rowtmp", name="row1")
            nc.vector.tensor_scalar(out=row1[:], in0=psm_i[:],
                                    scalar1=-1.0, scalar2=None, op0=OP.mult)
            nc.sync.dma_start(M3[1:2, :], row1[:])
            nc.sync.dma_start(M3[2:3, :], ones_bc[:])
            _stk2.close()

            # ---- Phase C: closed-form 2x2 inverse sqrt, fold gamma -> A
            def stile(tag):
                return small.tile([128, NFT], FP, tag=tag, name=tag)

            arr, bri, cii = stile("arr"), stile("bri"), stile("cii")
            nc.vector.tensor_scalar(out=arr[:], in0=cov[:, 0:NFT],
                                    scalar1=EPS, scalar2=None, op0=OP.add)
            nc.vector.tensor_copy(bri[:], cov[:, NFT:2 * NFT])
            nc.vector.tensor_scalar(out=cii[:], in0=cov[:, 2 * NFT:3 * NFT],
                                    scalar1=EPS, scalar2=None, op0=OP.add)

            det, tmp = stile("det"), stile("tmp")
            nc.vector.tensor_tensor(out=det[:], in0=arr[:], in1=cii[:],
                                    op=OP.mult)
            nc.vector.tensor_tensor(out=tmp[:], in0=bri[:], in1=bri[:],
                                    op=OP.mult)
            nc.vector.tensor_tensor(out=det[:], in0=det[:], in1=tmp[:],
                                    op=OP.subtract)
            s_t = stile("s_t")
            nc.scalar.activation(s_t[:], det[:], AF.Sqrt)
            # tval = sqrt(a + c + 2 s)
            tsum = stile("tsum")
            nc.vector.tensor_tensor(out=tsum[:], in0=arr[:], in1=cii[:],
                                    op=OP.add)
            nc.vector.scalar_tensor_tensor(out=tsum[:], in0=s_t[:], scalar=2.0,
                                           in1=tsum[:], op0=OP.mult, op1=OP.add)
            tval = stile("tval")
            nc.scalar.activation(tval[:], tsum[:], AF.Sqrt)
            den, rden = stile("den"), stile("rden")
            nc.vector.tensor_tensor(out=den[:], in0=s_t[:], in1=tval[:],
                                    op=OP.mult)
            nc.vector.reciprocal(rden[:], den[:])

            w_rr, w_ii, wri_n = stile("w_rr"), stile("w_ii"), stile("wri_n")
            # w_rr = (c+s)*rden ; w_ii = (a+s)*rden ; w_ri = -b*rden = wri_n
            nc.vector.tensor_tensor(out=w_rr[:], in0=cii[:], in1=s_t[:],
                                    op=OP.add)
            nc.vector.tensor_tensor(out=w_rr[:], in0=w_rr[:], in1=rden[:],
                                    op=OP.mult)
            nc.vector.tensor_tensor(out=w_ii[:], in0=arr[:], in1=s_t[:],
                                    op=OP.add)
            nc.vector.tensor_tensor(out=w_ii[:], in0=w_ii[:], in1=rden[:],
                                    op=OP.mult)
            nc.vector.tensor_tensor(out=wri_n[:], in0=bri[:], in1=rden[:],
                                    op=OP.mult)
            nc.vector.tensor_scalar(out=wri_n[:], in0=wri_n[:], scalar1=-1.0,
                                    scalar2=None, op0=OP.mult)

            # A = G @ W,  G = [[g_r, -g_i], [g_i, g_r]], W = [[w_rr, w_ri],
            # [w_ri, w_ii]] with w_ri = wri_n
            a_rr, a_ri = stile("a_rr"), stile("a_ri")
            a_ir, a_ii = stile("a_ir"), stile("a_ii")
            u, v = stile("u"), stile("v")
            # a_rr = g_r*w_rr - g_i*w_ri
            nc.vector.tensor_tensor(out=u[:], in0=g_r_t[:], in1=w_rr[:],
                                    op=OP.mult)
            nc.vector.tensor_tensor(out=v[:], in0=g_i_t[:], in1=wri_n[:],
                                    op=OP.mult)
            nc.vector.tensor_tensor(out=a_rr[:], in0=u[:], in1=v[:],
                                    op=OP.subtract)
            # a_ri = g_r*w_ri - g_i*w_ii
            nc.vector.tensor_tensor(out=u[:], in0=g_r_t[:], in1=wri_n[:],
                                    op=OP.mult)
            nc.vector.tensor_tensor(out=v[:], in0=g_i_t[:], in1=w_ii[:],
                                    op=OP.mult)
            nc.vector.tensor_tensor(out=a_ri[:], in0=u[:], in1=v[:],
                                    op=OP.subtract)
            # a_ir = g_i*w_rr + g_r*w_ri
            nc.vector.tensor_tensor(out=u[:], in0=g_i_t[:], in1=w_rr[:],
                                    op=OP.mult)
            nc.vector.tensor_tensor(out=v[:], in0=g_r_t[:], in1=wri_n[:],
                                    op=OP.mult)
            nc.vector.tensor_tensor(out=a_ir[:], in0=u[:], in1=v[:],
                                    op=OP.add)
            # a_ii = g_i*w_ri + g_r*w_ii
            nc.vector.tensor_tensor(out=u[:], in0=g_i_t[:], in1=wri_n[:],
                                    op=OP.mult)
            nc.vector.tensor_tensor(out=v[:], in0=g_r_t[:], in1=w_ii[:],
                                    op=OP.mult)
            nc.vector.tensor_tensor(out=a_ii[:], in0=u[:], in1=v[:],
                                    op=OP.add)

            # ---- A3C rhs for the K=3 correction matmul: (3, 4096)
            # row0[2f+c] = (a_rr, a_ir)[c][f]; row1: (a_ri, a_ii); row2: beta
            A3C = small.tile([3, 2 * F], FPR, tag="A3C")
            for row, (ev, od) in enumerate(((a_rr, a_ir), (a_ri, a_ii))):
                for cpar, srctile in ((0, ev), (1, od)):
                    # bounce through DRAM; read back in f-major order with a
                    # strided AP.  dram layout: addr(p, t) = 16*p + t.
                    dbuf = dram.tile([128, NFT], FPR, tag=f"dbuf{row}{cpar}",
                                     name=f"dbuf{row}{cpar}")
                    nc.sync.dma_start(dbuf[:], srctile[:].bitcast(FPR))
                    # src iterates (t, p): steps [[1, 16], [16, 128]]
                    src = dbuf[:].rearrange("p t -> (p t)").rearrange(
                        "(p t) -> t p", p=128, t=NFT
                    )
                    dst = A3C[row:row + 1, cpar::2].rearrange(
                        "z (t p) -> z t p", t=NFT, p=128
                    )
                    nc.sync.dma_start(dst, src)
            nc.sync.dma_start(A3C[2:3, :], beta_ilv[:])

            # ---- Phase D: apply.  t-outer; W built on the fly.
            _stk3 = ExitStack()
            ps_o = _stk3.enter_context(
                tc.tile_pool(name="ps_o", bufs=4, space="PSUM"))
            for t2 in range(NFT // 2):
                ta, tb = 2 * t2, 2 * t2 + 1
                Ws = []
                for t in (ta, tb):
                    W_r = wpool.tile([128, 256], FPR, tag="W_r",
                                     name=f"W_r_{t}")
                    W_i = wpool.tile([128, 256], FPR, tag="W_i",
                                     name=f"W_i_{t}")
                    for W, (ev, od) in ((W_r, (a_rr, a_ir)),
                                        (W_i, (a_ri, a_ii))):
                        Wv = W[:].rearrange("p (g c) -> p g c", c=2)
                        nc.vector.tensor_scalar(
                            out=Wv[:, :, 0], in0=ident_t[:],
                            scalar1=ev[:, t:t + 1], scalar2=None, op0=OP.mult,
                        )
                        nc.vector.tensor_scalar(
                            out=Wv[:, :, 1], in0=ident_t[:],
                            scalar1=od[:, t:t + 1], scalar2=None, op0=OP.mult,
                        )
                    Ws.append((W_r, W_i))
                for bh in range(2):
                    stg = stage.tile([128, 4 * 512], FP, tag="stg")
                    for bb in range(4):
                        b = 4 * bh + bb
                        po = ps_o.tile([128, 512], FP, tag="po")
                        # one accumulation group: M3 correction first
                        # (start=True over the full tile), then x terms.
                        mmr(
                            nc,
                            po[:],
                            M3[:, 128 * b:128 * (b + 1)],
                            A3C[:, 512 * t2:512 * (t2 + 1)],
                            start=True, stop=False,
                        )
                        for j, t in enumerate((ta, tb)):
                            W_r, W_i = Ws[j]
                            sl = slice(BC * t + 128 * b,
                                       BC * t + 128 * (b + 1))
                            mmr(
                                nc,
                                po[:, 256 * j:256 * (j + 1)],
                                xT_r[:, sl], W_r[:],
                                start=False, stop=False,
                            )
                            mmr(
                                nc,
                                po[:, 256 * j:256 * (j + 1)],
                                xT_i[:, sl], W_i[:],
                                start=False, stop=(j == 1),
                            )
                        if b % 2 == 0:
                            nc.vector.tensor_copy(
                                stg[:, 512 * bb:512 * (bb + 1)], po[:])
                        else:
                            nc.scalar.copy(
                                stg[:, 512 * bb:512 * (bb + 1)], po[:])
                    # 1 MB store: rows (b, p) -> out[128*b + p, 512*t2:+512]
                    # alternate stores across the two HWDGE rings
                    dst = out.rearrange("(a p) f -> p a f", p=128)[
                        :, 4 * bh:4 * (bh + 1), 512 * t2:512 * (t2 + 1)
                    ]
                    src = stg[:].rearrange("p (a q) -> p a q", q=512)
                    if (2 * t2 + bh) % 2 == 0:
                        nc.sync.dma_start(dst, src)
                    else:
                        nc.scalar.dma_start(dst, src)
            _stk3.close()

    split_multi_waits(nc)
    return nc


_CACHE = {}


def _get_nc():
    if "nc" not in _CACHE:
        _CACHE["nc"] = build_bass()
    return _CACHE["nc"]


def _constants():
    if "consts" not in _CACHE:
        sel = np.zeros((128, CSH), dtype=np.float32)
        for p in range(128):
            sel[p, p % CSH] = 1.0
        _CACHE["consts"] = {
            "ident": np.eye(128, dtype=np.float32),
            "identsel": np.ascontiguousarray(
                np.concatenate([np.eye(128, dtype=np.float32), sel], axis=1)),
            "onesF": np.full((128, 1), 1.0 / F, dtype=np.float32),
            "ones_bc": np.ones((1, BC), dtype=np.float32),
        }
    return _CACHE["consts"]


def kernel(x_real, x_imag, gamma_r, gamma_i, beta_r, beta_i):
    x_real = np.ascontiguousarray(x_real, dtype=np.float32)
    x_imag = np.ascontiguousarray(x_imag, dtype=np.float32)
    gamma_r = np.asarray(gamma_r, dtype=np.float32)
    gamma_i = np.asarray(gamma_i, dtype=np.float32)
    beta_r = np.asarray(beta_r, dtype=np.float32)
    beta_i = np.asarray(beta_i, dtype=np.float32)

    nc = _get_nc()
    consts = _constants()
    g_r_t = np.ascontiguousarray(gamma_r.reshape(NFT, 128).T)
    g_i_t = np.ascontiguousarray(gamma_i.reshape(NFT, 128).T)
    beta_ilv = np.ascontiguousarray(
        np.stack([beta_r, beta_i], axis=-1).reshape(1, 2 * F)
    )

    in_maps = []
    for k in range(NCORES):
        cs = slice(CSH * k, CSH * (k + 1))
        in_maps.append({
            "x_r": np.ascontiguousarray(
                x_real[:, cs, :].reshape(BC, F)),
            "x_i": np.ascontiguousarray(
                x_imag[:, cs, :].reshape(BC, F)),
            "g_r": g_r_t, "g_i": g_i_t, "beta_ilv": beta_ilv,
            **consts,
        })

    res = run_bass_kernel_spmd(nc, in_maps, list(range(NCORES)))

    full = np.empty((B, C, F, 2), dtype=np.float32)
    for k in range(NCORES):
        full[:, CSH * k:CSH * (k + 1)] = (
            res.results[k]["out"].reshape(B, CSH, F, 2)
        )
    return full



# revision 17
# speedup vs baseline: 1.2195x; 1.2195x over previous
"""ComplexLayerNorm Trainium2 kernel (8 NeuronCores, SPMD, C-sharded).

Math (see reference): per-feature 2x2 covariance whitening of (re, im) over
all B*C samples (centered with the batch-only mean mu_b), after subtracting
the complex mean over F, plus complex affine.

Sharding: C (=128) split 16-per-core, so the batch-sums T[c,f] = sum_b x and
mu_b are core-local and only 3 partial second-moment F-vectors (24 KB) need a
cross-core sum.

Per-core dataflow (v2):
  1. Load x natural ((bc=1024) x (f=2048)) as float32r; one fused fp32r
     matmul per f-chunk against [I | sel | 0] (256 wide so the PE runs at
     1 cycle/row) yields the 128-col transpose block AND the 16-col T
     partial.  PSUM->SBUF copies store xT in bf16.
  2. Mean over F rides phase A: PE (-1/F)-ones matmuls accumulate -mean into
     PSUM per bc-half as the halves complete (b=3 / b=7).
  3. Second moments per f: STT/Square ops with accum_out, greedy-balanced
     over DVE/Pool/ACT, fired incrementally (bc 0:768 while b=6,7 stream,
     768:1024 as a short tail).  T-quadratic correction likewise; partial
     cov -> AllReduce (24 KB).
  4. Closed-form 2x2 inverse sqrt; fold complex gamma -> A (2x2 per f).
  5. Apply on PE: bf16 diag-W matmuls (transpose-back + scale in one) plus
     one fp32r K=3 matmul adding (-A@mean + beta).  Output lands interleaved
     (f, 2) -- exactly the (B,C,F,2) layout.
Host only slices inputs per core and concatenates the 8 C-shards.
"""

import numpy as np

import bass_rust
import concourse.bass as bass
import concourse.mybir as mybir
from concourse import tile
from concourse.bass_utils import run_bass_kernel_spmd


def split_multi_waits(nc):
    """The walrus build in this container allows only ONE sync-wait command
    per instruction; Tile emits several.  Split extras into preceding
    single-wait NoOps on the same engine (sequential waits == AND)."""
    cnt = 0
    for bb in nc.main_func.blocks:
        il = bb.instructions
        newlist = []
        changed = False
        for inst in list(il):
            si = inst.sync_info
            waits = list(si.on_wait) if si else []
            if len(waits) > 1:
                changed = True
                for w in waits[:-1]:
                    cnt += 1
                    nop = bass_rust.InstNoOp(name=f"I-wsplit-{cnt}")
                    nop.engine = inst.engine
                    nop.sync_info = mybir.SyncInfo(on_wait=[w], on_update=[])
                    newlist.append(nop)
                inst.sync_info = mybir.SyncInfo(
                    on_wait=[waits[-1]], on_update=list(si.on_update))
            newlist.append(inst)
        if changed:
            il[:] = newlist
    return cnt


FP = mybir.dt.float32
FPR = mybir.dt.float32r
BF = mybir.dt.bfloat16
AF = mybir.ActivationFunctionType
OP = mybir.AluOpType

B, C, F = 64, 128, 2048
NCORES = 8
CSH = C // NCORES           # 16 channels per core
BC = B * CSH                # 1024 sample rows per core
NFT = F // 128              # 16 f-chunks
NBC = BC // 128             # 8 bc-chunks
EPS = 1e-4
NM1 = float(B * C - 1)      # 8191


class Greedy:
    """Greedy engine load balancer for PSUM-copy / stats ops."""

    def __init__(self, nc):
        self.nc = nc
        self.load = {"dve": 0.0, "act": 0.0, "pool": 0.0}
        self.eng = {"dve": nc.vector, "act": nc.scalar, "pool": nc.gpsimd}

    def pick(self, costs):
        """costs: dict engine -> ns (missing = ineligible)."""
        name = min(costs, key=lambda k: self.load[k] + costs[k])
        self.load[name] += costs[name]
        return name, self.eng[name]

    def bump(self, name, ns):
        self.load[name] += ns

    def copy(self, costs, dst, src):
        name, eng = self.pick(costs)
        if name == "act":
            eng.copy(dst, src)
        elif name == "pool":
            eng.dma_start(dst, src)
        else:
            eng.tensor_copy(dst, src)


def build_bass():
    nc = bass.Bass()

    x_r = nc.dram_tensor("x_r", [BC, F], FPR, kind="ExternalInput")
    x_i = nc.dram_tensor("x_i", [BC, F], FPR, kind="ExternalInput")
    # gamma pre-tiled on host to (128, NFT): tile[p, t] = gamma[128*t + p]
    g_r = nc.dram_tensor("g_r", [128, NFT], FP, kind="ExternalInput")
    g_i = nc.dram_tensor("g_i", [128, NFT], FP, kind="ExternalInput")
    # beta interleaved on host: (1, 4096) = [b_r[0], b_i[0], b_r[1], ...]
    beta_ilv = nc.dram_tensor("beta_ilv", [1, 2 * F], FPR, kind="ExternalInput")
    ident_bf = nc.dram_tensor("ident_bf", [128, 128], BF, kind="ExternalInput")
    # [I | sel | zeros]: 256 wide so the fp32r fused matmul runs 1 cyc/row
    identsel = nc.dram_tensor("identsel", [128, 256], FPR,
                              kind="ExternalInput")
    onesF = nc.dram_tensor("onesF", [128, 1], BF, kind="ExternalInput")
    ones_bc = nc.dram_tensor("ones_bc", [1, BC], FPR, kind="ExternalInput")

    out = nc.dram_tensor("out", [BC, 2 * F], FP, kind="ExternalOutput")

    gd = None

    with tile.TileContext(nc) as tc:
        with (
            tc.tile_pool(name="big", bufs=1) as big,
            tc.tile_pool(name="small", bufs=1) as small,
            tc.tile_pool(name="wpool", bufs=2) as wpool,
            tc.tile_pool(name="stage", bufs=2) as stage,
            tc.tile_pool(name="dram", bufs=1, space="DRAM") as dram,
        ):
            gd = Greedy(nc)

            # ---- constants to SBUF
            ident_t = small.tile([128, 128], BF, tag="ident")
            nc.sync.dma_start(ident_t[:], ident_bf[:])
            identsel_t = small.tile([128, 256], FPR, tag="identsel")
            nc.sync.dma_start(identsel_t[:], identsel[:])
            onesF_t = small.tile([128, 1], BF, tag="onesF")
            nc.sync.dma_start(onesF_t[:], onesF[:])
            g_r_t = small.tile([128, NFT], FP, tag="g_r")
            nc.sync.dma_start(g_r_t[:], g_r[:])
            g_i_t = small.tile([128, NFT], FP, tag="g_i")
            nc.sync.dma_start(g_i_t[:], g_i[:])

            # ---- persistent: x transposed (bf16),
            # xT[p, 1024*t + j] = x[j, 128*t+p]
            xT_r = big.tile([128, NFT * BC], BF, tag="xT_r")
            xT_i = big.tile([128, NFT * BC], BF, tag="xT_i")

            from contextlib import ExitStack
            _stk = ExitStack()
            xin = _stk.enter_context(tc.tile_pool(name="xin", bufs=3))
            scratch = _stk.enter_context(tc.tile_pool(name="scratch", bufs=2))
            ps_xt = _stk.enter_context(
                tc.tile_pool(name="ps_xt", bufs=2, space="PSUM"))
            ps_mean = _stk.enter_context(
                tc.tile_pool(name="ps_mean", bufs=1, space="PSUM"))

            # T[c,f] accumulators in SBUF: [p (=f in chunk), 16*t + c]
            T_r_sb = small.tile([128, NFT * CSH], FP, tag="T_r_sb")
            T_i_sb = small.tile([128, NFT * CSH], FP, tag="T_i_sb")

            # -mean accumulators (PSUM), one 512-wide accum group per bank
            psm_r = ps_mean.tile([1, BC], FP, tag="psm_r")
            psm_i = ps_mean.tile([1, BC], FP, tag="psm_i")

            # stats accum slots: cols q*16 + t, q in (rr, ri, ii)
            S_t = small.tile([128, 3 * NFT], FP, tag="S_t")

            # pre-warm the ACT Square table so the load is off the tail
            warm = small.tile([128, 16], FP, tag="warm")
            nc.vector.memset(warm[:], 0.0)
            warm2 = small.tile([128, 16], FP, tag="warm2")
            nc.scalar.activation(warm2[:], warm[:], AF.Square)

            # walrus: Pool (gpsimd) has no PSUM access and no ALU STT;
            # it CAN issue casting SWDGE DMAs, so it moves PSUM blocks.
            COST = {
                "copy": {"dve": 660, "act": 713},
                "tacc": {"dve": 260},
                "stat": {"dve": 1190, "act": 1225},  # full 1024-col slab
            }

            def emit_stat(q, t, accum):
                """One second-moment op over slab xT[:, 1024t : 1024(t+1)]."""
                sl = slice(BC * t, BC * (t + 1))
                costs = dict(COST["stat"])
                if q == "ri":
                    costs.pop("act", None)
                name, eng = gd.pick(costs)
                scr = scratch.tile([128, BC], BF, tag="scr",
                                   name=f"scr_{q}_{t}_{name}")
                in0 = xT_r[:, sl] if q in ("rr", "ri") else xT_i[:, sl]
                in1 = xT_i[:, sl] if q in ("ri", "ii") else xT_r[:, sl]
                if name == "act":
                    eng.activation(scr[:], in0, AF.Square, accum_out=accum)
                else:
                    eng.scalar_tensor_tensor(
                        out=scr[:], in0=in0, scalar=1.0, in1=in1,
                        op0=OP.mult, op1=OP.mult, accum_out=accum)

            QIDX = {"rr": 0, "ri": 1, "ii": 2}

            # ---- Phase A: load natural; fused PE transpose + T; mean; stats
            for b in range(NBC):
                xn_r = xin.tile([128, F], FPR, tag="xn", name=f"xn_r{b}")
                nc.sync.dma_start(xn_r[:], x_r[128 * b:128 * (b + 1), :])
                xn_i = xin.tile([128, F], FPR, tag="xn", name=f"xn_i{b}")
                nc.gpsimd.dma_start(xn_i[:], x_i[128 * b:128 * (b + 1), :])
                for ri, (xn, xT, T_sb) in enumerate(
                    ((xn_r, xT_r, T_r_sb), (xn_i, xT_i, T_i_sb))
                ):
                    for tg in range(NFT // 4):
                        pxt = ps_xt.tile([128, 1024], FP, tag="pxt")
                        for tt in range(4):
                            t = 4 * tg + tt
                            nc.tensor.matmul(
                                pxt[:, 256 * tt:256 * (tt + 1)],
                                xn[:, 128 * t:128 * (t + 1)],
                                identsel_t[:],
                                start=True, stop=True,
                            )
                        pv = pxt[:].rearrange("p (a q) -> p a q", q=256)
                        dst = xT[:].rearrange("p (a q) -> p a q", q=1024)[
                            :, 4 * tg:4 * (tg + 1), 128 * b:128 * (b + 1)
                        ]
                        gd.copy(COST["copy"], dst, pv[:, :, 0:128])
                        # T partial accumulate
                        tdst = T_sb[:, 64 * tg:64 * (tg + 1)].rearrange(
                            "p (a q) -> p a q", q=CSH)
                        tsrc = pv[:, :, 128:128 + CSH]
                        if b == 0:
                            gd.copy({k: 420 for k in COST["tacc"]},
                                    tdst, tsrc)
                        else:
                            _, eng = gd.pick(COST["tacc"])
                            eng.scalar_tensor_tensor(
                                out=tdst, in0=tsrc, scalar=1.0, in1=tdst,
                                op0=OP.mult, op1=OP.add,
                            )
                # -mean accumulation per bc-half on PE (xT is complete for
                # cols [512h : 512h+512) of every chunk after b = 4h+3)
                if b in (3, 7):
                    h = b // 4
                    for xT, psm in ((xT_r, psm_r), (xT_i, psm_i)):
                        for t in range(NFT):
                            nc.tensor.matmul(
                                psm[:, 512 * h:512 * (h + 1)],
                                onesF_t[:],
                                xT[:, BC * t + 512 * h:
                                   BC * t + 512 * (h + 1)],
                                start=(t == 0), stop=(t == NFT - 1),
                            )
            # second moments, one full slab per (quantity, chunk)
            for t in range(NFT):
                for q in ("rr", "ri", "ii"):
                    emit_stat(q, t,
                              S_t[:, QIDX[q] * NFT + t:QIDX[q] * NFT + t + 1])

            # ---- T quadratic correction: corr[:, q*16+t] = sum_c T_x*T_y
            corr = small.tile([128, 3 * NFT], FP, tag="corr")
            for t in range(NFT):
                sl = slice(CSH * t, CSH * (t + 1))
                for q, (Ta, Tb) in (("rr", (T_r_sb, T_r_sb)),
                                    ("ri", (T_r_sb, T_i_sb)),
                                    ("ii", (T_i_sb, T_i_sb))):
                    gd.bump("dve", 120)
                    ts = scratch.tile([128, CSH], FP, tag="tsq",
                                      name=f"tsq_{q}_{t}")
                    nc.vector.scalar_tensor_tensor(
                        out=ts[:], in0=Ta[:, sl], scalar=1.0, in1=Tb[:, sl],
                        op0=OP.mult, op1=OP.mult,
                        accum_out=corr[:, QIDX[q] * NFT + t:
                                       QIDX[q] * NFT + t + 1])

            # ---- local partial covariance: (S_a + S_b - corr/B) / (n-1)
            partial = small.tile([128, 3 * NFT], FP, tag="partial")
            nc.vector.scalar_tensor_tensor(
                out=partial[:], in0=corr[:], scalar=-1.0 / B, in1=S_t[:],
                op0=OP.mult, op1=OP.add)
            nc.vector.tensor_scalar(
                out=partial[:], in0=partial[:], scalar1=1.0 / NM1,
                scalar2=None, op0=OP.mult)

            # ---- AllReduce partial covariance (24 KB)
            ar_in = dram.tile([128, 3 * NFT], FP, tag="ar_in")
            ar_out = dram.tile([128, 3 * NFT], FP, tag="ar_out")
            nc.sync.dma_start(ar_in[:], partial[:])
            nc.gpsimd.collective_compute(
                "AllReduce", OP.add,
                replica_groups=[list(range(NCORES))],
                ins=[ar_in.opt()],
                outs=[ar_out.opt()],
            )
            cov = small.tile([128, 3 * NFT], FP, tag="cov")
            nc.sync.dma_start(cov[:], ar_out[:])

            # ---- M3 = [-mean_r; -mean_i; ones] (3, 1024) via DMA bounce
            M3 = small.tile([3, BC], FPR, tag="M3")
            row0 = small.tile([1, BC], FPR, tag="rowtmp", name="row0")
            nc.vector.tensor_copy(row0[:], psm_r[:])
            nc.sync.dma_start(M3[0:1, :], row0[:])
            row1 = small.tile([1, BC], FPR, tag="rowtmp", name="row1")
            nc.vector.tensor_copy(row1[:], psm_i[:])
            nc.sync.dma_start(M3[1:2, :], row1[:])
            nc.sync.dma_start(M3[2:3, :], ones_bc[:])

            _stk.close()

            # ---- closed-form 2x2 inverse sqrt, fold gamma -> A
            def stile(tag):
                return small.tile([128, NFT], FP, tag=tag, name=tag)

            arr, bri, cii = stile("arr"), stile("bri"), stile("cii")
            nc.vector.tensor_scalar(out=arr[:], in0=cov[:, 0:NFT],
                                    scalar1=EPS, scalar2=None, op0=OP.add)
            nc.vector.tensor_copy(bri[:], cov[:, NFT:2 * NFT])
            nc.vector.tensor_scalar(out=cii[:], in0=cov[:, 2 * NFT:3 * NFT],
                                    scalar1=EPS, scalar2=None, op0=OP.add)

            det, tmp = stile("det"), stile("tmp")
            nc.vector.tensor_tensor(out=det[:], in0=arr[:], in1=cii[:],
                                    op=OP.mult)
            nc.vector.tensor_tensor(out=tmp[:], in0=bri[:], in1=bri[:],
                                    op=OP.mult)
            nc.vector.tensor_tensor(out=det[:], in0=det[:], in1=tmp[:],
                                    op=OP.subtract)
            s_t = stile("s_t")
            nc.scalar.activation(s_t[:], det[:], AF.Sqrt)
            tsum = stile("tsum")
            nc.vector.tensor_tensor(out=tsum[:], in0=arr[:], in1=cii[:],
                                    op=OP.add)
            nc.vector.scalar_tensor_tensor(out=tsum[:], in0=s_t[:], scalar=2.0,
                                           in1=tsum[:], op0=OP.mult,
                                           op1=OP.add)
            tval = stile("tval")
            nc.scalar.activation(tval[:], tsum[:], AF.Sqrt)
            den, rden = stile("den"), stile("rden")
            nc.vector.tensor_tensor(out=den[:], in0=s_t[:], in1=tval[:],
                                    op=OP.mult)
            nc.vector.reciprocal(rden[:], den[:])

            w_rr, w_ii, wri_n = stile("w_rr"), stile("w_ii"), stile("wri_n")
            nc.vector.tensor_tensor(out=w_rr[:], in0=cii[:], in1=s_t[:],
                                    op=OP.add)
            nc.vector.tensor_tensor(out=w_rr[:], in0=w_rr[:], in1=rden[:],
                                    op=OP.mult)
            nc.vector.tensor_tensor(out=w_ii[:], in0=arr[:], in1=s_t[:],
                                    op=OP.add)
            nc.vector.tensor_tensor(out=w_ii[:], in0=w_ii[:], in1=rden[:],
                                    op=OP.mult)
            nc.vector.tensor_tensor(out=wri_n[:], in0=bri[:], in1=rden[:],
                                    op=OP.mult)
            nc.vector.tensor_scalar(out=wri_n[:], in0=wri_n[:], scalar1=-1.0,
                                    scalar2=None, op0=OP.mult)

            # A = G @ W, G = [[g_r, -g_i], [g_i, g_r]],
            # W = [[w_rr, w_ri], [w_ri, w_ii]], w_ri = wri_n
            a_rr, a_ri = stile("a_rr"), stile("a_ri")
            a_ir, a_ii = stile("a_ir"), stile("a_ii")
            u, v = stile("u"), stile("v")
            nc.vector.tensor_tensor(out=u[:], in0=g_r_t[:], in1=w_rr[:],
                                    op=OP.mult)
            nc.vector.tensor_tensor(out=v[:], in0=g_i_t[:], in1=wri_n[:],
                                    op=OP.mult)
            nc.vector.tensor_tensor(out=a_rr[:], in0=u[:], in1=v[:],
                                    op=OP.subtract)
            nc.vector.tensor_tensor(out=u[:], in0=g_r_t[:], in1=wri_n[:],
                                    op=OP.mult)
            nc.vector.tensor_tensor(out=v[:], in0=g_i_t[:], in1=w_ii[:],
                                    op=OP.mult)
            nc.vector.tensor_tensor(out=a_ri[:], in0=u[:], in1=v[:],
                                    op=OP.subtract)
            nc.vector.tensor_tensor(out=u[:], in0=g_i_t[:], in1=w_rr[:],
                                    op=OP.mult)
            nc.vector.tensor_tensor(out=v[:], in0=g_r_t[:], in1=wri_n[:],
                                    op=OP.mult)
            nc.vector.tensor_tensor(out=a_ir[:], in0=u[:], in1=v[:],
                                    op=OP.add)
            nc.vector.tensor_tensor(out=u[:], in0=g_i_t[:], in1=wri_n[:],
                                    op=OP.mult)
            nc.vector.tensor_tensor(out=v[:], in0=g_r_t[:], in1=w_ii[:],
                                    op=OP.mult)
            nc.vector.tensor_tensor(out=a_ii[:], in0=u[:], in1=v[:],
                                    op=OP.add)

            # ---- A3C rhs for the K=3 correction matmul: (3, 4096)
            A3C = small.tile([3, 2 * F], FPR, tag="A3C")
            for row, (ev, od) in enumerate(((a_rr, a_ir), (a_ri, a_ii))):
                for cpar, srctile in ((0, ev), (1, od)):
                    dbuf = dram.tile([128, NFT], FPR, tag=f"dbuf{row}{cpar}",
                                     name=f"dbuf{row}{cpar}")
                    nc.sync.dma_start(dbuf[:], srctile[:].bitcast(FPR))
                    src = dbuf[:].rearrange("p t -> (p t)").rearrange(
                        "(p t) -> t p", p=128, t=NFT
                    )
                    dst = A3C[row:row + 1, cpar::2].rearrange(
                        "z (t p) -> z t p", t=NFT, p=128
                    )
                    if (row + cpar) % 2 == 0:
                        nc.sync.dma_start(dst, src)
                    else:
                        nc.scalar.dma_start(dst, src)
            nc.sync.dma_start(A3C[2:3, :], beta_ilv[:])

            # ---- apply: diag matmuls (bf16) first, K=3 correction closes
            from contextlib import ExitStack as ES2
            _stk3 = ES2()
            ps_o = _stk3.enter_context(
                tc.tile_pool(name="ps_o", bufs=4, space="PSUM"))
            store_k = 0
            for t2 in range(NFT // 2):
                ta, tb = 2 * t2, 2 * t2 + 1
                Ws = []
                for t in (ta, tb):
                    W_r = wpool.tile([128, 256], BF, tag="W_r",
                                     name=f"W_r_{t}")
                    W_i = wpool.tile([128, 256], BF, tag="W_i",
                                     name=f"W_i_{t}")
                    for W, (ev, od) in ((W_r, (a_rr, a_ir)),
                                        (W_i, (a_ri, a_ii))):
                        Wv = W[:].rearrange("p (g c) -> p g c", c=2)
                        nc.vector.tensor_scalar(
                            out=Wv[:, :, 0], in0=ident_t[:],
                            scalar1=ev[:, t:t + 1], scalar2=None, op0=OP.mult,
                        )
                        nc.vector.tensor_scalar(
                            out=Wv[:, :, 1], in0=ident_t[:],
                            scalar1=od[:, t:t + 1], scalar2=None, op0=OP.mult,
                        )
                    Ws.append((W_r, W_i))
                for bh in range(2):
                    stg = stage.tile([128, 4 * 512], FP, tag="stg")
                    for bb in range(4):
                        b = 4 * bh + bb
                        po = ps_o.tile([128, 512], FP, tag="po")
                        for j, t in enumerate((ta, tb)):
                            W_r, W_i = Ws[j]
                            sl = slice(BC * t + 128 * b,
                                       BC * t + 128 * (b + 1))
                            nc.tensor.matmul(
                                po[:, 256 * j:256 * (j + 1)],
                                xT_r[:, sl], W_r[:],
                                start=(j == 0), stop=False,
                            )
                            nc.tensor.matmul(
                                po[:, 256 * j:256 * (j + 1)],
                                xT_i[:, sl], W_i[:],
                                start=False, stop=False,
                            )
                        nc.tensor.matmul(
                            po[:],
                            M3[:, 128 * b:128 * (b + 1)],
                            A3C[:, 512 * t2:512 * (t2 + 1)],
                            start=False, stop=True,
                        )
                        gd.copy(COST["copy"],
                                stg[:, 512 * bb:512 * (bb + 1)], po[:])
                    dst = out.rearrange("(a p) f -> p a f", p=128)[
                        :, 4 * bh:4 * (bh + 1), 512 * t2:512 * (t2 + 1)
                    ]
                    src = stg[:].rearrange("p (a q) -> p a q", q=512)
                    if store_k % 2 == 0:
                        nc.sync.dma_start(dst, src)
                    else:
                        nc.gpsimd.dma_start(dst, src)
                    store_k += 1
            _stk3.close()

    split_multi_waits(nc)
    return nc


_CACHE = {}


def _get_nc():
    if "nc" not in _CACHE:
        _CACHE["nc"] = build_bass()
    return _CACHE["nc"]


def _constants():
    if "consts" not in _CACHE:
        import ml_dtypes
        sel = np.zeros((128, CSH), dtype=np.float32)
        for p in range(128):
            sel[p, p % CSH] = 1.0
        identsel = np.zeros((128, 256), dtype=np.float32)
        identsel[:, 0:128] = np.eye(128, dtype=np.float32)
        identsel[:, 128:128 + CSH] = sel
        _CACHE["consts"] = {
            "ident_bf": np.eye(128, dtype=ml_dtypes.bfloat16),
            "identsel": identsel,
            "onesF": np.full((128, 1), -1.0 / F, dtype=ml_dtypes.bfloat16),
            "ones_bc": np.ones((1, BC), dtype=np.float32),
        }
    return _CACHE["consts"]


def kernel(x_real, x_imag, gamma_r, gamma_i, beta_r, beta_i):
    x_real = np.ascontiguousarray(x_real, dtype=np.float32)
    x_imag = np.ascontiguousarray(x_imag, dtype=np.float32)
    gamma_r = np.asarray(gamma_r, dtype=np.float32)
    gamma_i = np.asarray(gamma_i, dtype=np.float32)
    beta_r = np.asarray(beta_r, dtype=np.float32)
    beta_i = np.asarray(beta_i, dtype=np.float32)

    nc = _get_nc()
    consts = _constants()
    g_r_t = np.ascontiguousarray(gamma_r.reshape(NFT, 128).T)
    g_i_t = np.ascontiguousarray(gamma_i.reshape(NFT, 128).T)
    beta_ilv = np.ascontiguousarray(
        np.stack([beta_r, beta_i], axis=-1).reshape(1, 2 * F)
    )

    in_maps = []
    for k in range(NCORES):
        cs = slice(CSH * k, CSH * (k + 1))
        in_maps.append({
            "x_r": np.ascontiguousarray(
                x_real[:, cs, :].reshape(BC, F)),
            "x_i": np.ascontiguousarray(
                x_imag[:, cs, :].reshape(BC, F)),
            "g_r": g_r_t, "g_i": g_i_t, "beta_ilv": beta_ilv,
            **consts,
        })

    res = run_bass_kernel_spmd(nc, in_maps, list(range(NCORES)))

    full = np.empty((B, C, F, 2), dtype=np.float32)
    for k in range(NCORES):
        full[:, CSH * k:CSH * (k + 1)] = (
            res.results[k]["out"].reshape(B, CSH, F, 2)
        )
    return full
